# revision 21
# baseline (speedup 1.0000x reference)
"""Trainium2 Bass kernel for DiscreteLSTMActor.

Strategy: data-parallel over batch (B=256 -> 32 per core x 8 cores).
Everything on-chip runs in a "transposed" layout: features on SBUF
partitions, samples along the free dim, so the LSTM gate elementwise
work uses all 128 lanes.

Per core pipeline:
  P1  encoder: PE-transpose obs tiles, featsT = relu(W_encT.T @ obsT + b),
      build xT = [featsT; clip(reward); one_hotT(last_action)]
  P2  ihpre0 = w_ih0'(permuted,transposed) @ xT + (b_ih0+b_hh0)  -> DRAM
  P3  layer-0 recurrence over T (only h-matmul per step, gate update in
      [128,32] chunk layout), h0 states -> DRAM
  P4  ihpre1 = w_ih1' @ h0T + biases -> DRAM
  P5  layer-1 recurrence, h1 states -> DRAM
  P6  heads: logitsT = headT.T @ h1T + b; argmax(logits+gumbel) on device
      (gumbel table for jax.random.key(1) is passed in as a constant)

Gate dimension (4H=2112) is permuted into 20 "pieces": 16 full pieces of
128 rows (gate-major: i0..i3 f0..f3 g0..g3 o0..o3) plus 4 tail pieces of
16 rows, so every elementwise slice is partition-aligned.

Host side only shards/replicates inputs, pre-packs weight layouts, and
re-assembles outputs.
"""

import os

import numpy as np

import concourse.bacc as bacc
import concourse.mybir as mybir
import concourse.tile as tile
from concourse.bass import ds
from concourse.bass_utils import run_bass_kernel_spmd
from concourse.masks import make_identity

F32 = mybir.dt.float32
I32 = mybir.dt.int32
U8 = mybir.dt.uint8
U32 = mybir.dt.uint32
F32R = mybir.dt.float32r
# fp32r (4x-rate fp32 PE path) crashes NRT on this stack; keep opt-in
USE_F32R = bool(int(os.environ.get("K_F32R", "0")))
RT = F32R if USE_F32R else F32
AF = mybir.ActivationFunctionType
ALU = mybir.AluOpType

T_FULL, B_FULL, OBS, FEAT, A = 256, 256, 512, 512, 15
H = FEAT + A + 1  # 528
G4 = 4 * H  # 2112
NCORES = 8
B_SH = B_FULL // NCORES  # 32
TS = 16  # timesteps per big-matmul chunk (chunk = TS*B_SH = 512 samples)

# k-chunks of the H=528 contraction dim
KC = [(0, 128), (128, 128), (256, 128), (384, 128), (512, 16)]
# gate pieces: (gate, chunk); 16 full + 4 tail
PIECES = [(g, c) for c in range(4) for g in range(4)] + [(g, 4) for g in range(4)]


def piece_meta(pi):
    g, c = PIECES[pi]
    msz = 128 if c < 4 else 16
    joff = pi * 128 if pi < 16 else 2048 + (pi - 16) * 16
    col = pi * 32  # column offset in the 640-wide gates/pre layout
    wrow = g * H + c * 128  # first row in the original [2112, 528] weight
    return msz, joff, col, wrow


def _jperm():
    idx = []
    for pi in range(20):
        msz, _, _, wrow = piece_meta(pi)
        idx.extend(range(wrow, wrow + msz))
    return np.asarray(idx, dtype=np.int64)


JPERM = _jperm()


def pack_wT(w):
    """w [2112, 528] -> [5, 128, 2112] (k-chunk, k-in-chunk, j')."""
    wp = np.asarray(w, np.float32)[JPERM, :]  # [2112, 528]
    out = np.zeros((5, 128, G4), np.float32)
    for kc, (off, ksz) in enumerate(KC):
        out[kc, :ksz, :] = wp[:, off:off + ksz].T
    return np.ascontiguousarray(out)


def pack_bias(b_ih, b_hh):
    bb = (np.asarray(b_ih, np.float32) + np.asarray(b_hh, np.float32))[JPERM]
    out = np.zeros((128, 20), np.float32)
    for pi in range(20):
        msz, joff, _, _ = piece_meta(pi)
        out[:msz, pi] = bb[joff:joff + msz]
    return np.ascontiguousarray(out)


# ---------------------------------------------------------------------------
# device program
# ---------------------------------------------------------------------------

def _r(ap):
    # fp32 tensors reinterpreted as float32r for the 4x-rate PE path on
    # big (N=512) matmuls; bit layout is identical
    return ap.bitcast(F32R) if USE_F32R else ap


def build_program(Tn):
    assert Tn % TS == 0
    NCH = Tn * B_SH // (TS * B_SH)  # number of 512-sample chunks = Tn // TS
    N = Tn * B_SH

    nc = bacc.Bacc("TRN2", target_bir_lowering=False)

    # inputs
    obs_d = nc.dram_tensor("obs", [Tn, B_SH, OBS], F32, kind="ExternalInput")
    la_d = nc.dram_tensor("la", [Tn, B_SH, 1], I32, kind="ExternalInput")
    rew_d = nc.dram_tensor("rew", [1, Tn, B_SH], F32, kind="ExternalInput")
    term_d = nc.dram_tensor("term", [1, Tn, B_SH], U8, kind="ExternalInput")
    gum_d = nc.dram_tensor("gum", [N, A], F32, kind="ExternalInput")
    wencT_d = nc.dram_tensor("wencT", [4, 128, FEAT], F32, kind="ExternalInput")
    bencT_d = nc.dram_tensor("bencT", [128, 4], F32, kind="ExternalInput")
    wih0_d = nc.dram_tensor("wih0T", [5, 128, G4], F32, kind="ExternalInput")
    whh0_d = nc.dram_tensor("whh0T", [5, 128, G4], F32, kind="ExternalInput")
    wih1_d = nc.dram_tensor("wih1T", [5, 128, G4], F32, kind="ExternalInput")
    whh1_d = nc.dram_tensor("whh1T", [5, 128, G4], F32, kind="ExternalInput")
    b0p_d = nc.dram_tensor("b0p", [128, 20], F32, kind="ExternalInput")
    b1p_d = nc.dram_tensor("b1p", [128, 20], F32, kind="ExternalInput")
    headT_d = nc.dram_tensor("headT", [5, 128, 16], F32, kind="ExternalInput")
    bhead_d = nc.dram_tensor("bhead", [16, 1], F32, kind="ExternalInput")

    # outputs
    polT_d = nc.dram_tensor("polT", [A, N], F32, kind="ExternalOutput")
    baseT_d = nc.dram_tensor("baseT", [1, N], F32, kind="ExternalOutput")
    act_d = nc.dram_tensor("act", [N, 1], U32, kind="ExternalOutput")
    hc_d = nc.dram_tensor("hc", [4, 128, 160], F32, kind="ExternalOutput")

    with tile.TileContext(nc) as tc:
        with (
            tc.tile_pool(name="dram", bufs=1, space="DRAM") as dpool,
            tc.tile_pool(name="const", bufs=1) as cpool,
            tc.tile_pool(name="wpool", bufs=1) as wpool,
            tc.tile_pool(name="state", bufs=1) as spool,
        ):
            pre0 = dpool.tile([Tn, 128, 640], F32)
            pre1 = dpool.tile([Tn, 128, 640], F32)
            h0a = dpool.tile([Tn, 128, 160], F32)
            h1a = dpool.tile([Tn, 128, 160], F32)

            ident = cpool.tile([128, 128], F32)
            make_identity(nc, ident)
            iota16i = cpool.tile([128, 16], I32)
            nc.gpsimd.iota(iota16i, pattern=[[1, 16]], base=-1,
                           channel_multiplier=0)
            iota16 = cpool.tile([128, 16], F32)
            nc.vector.tensor_copy(iota16, iota16i)

            # nd = 1 - terminated, with a trailing all-ones entry so the
            # recurrence can apply step t+1's mask at step t
            ndf = cpool.tile([1, (Tn + 1) * B_SH], F32)
            nc.gpsimd.dma_start(
                out=ndf[0:1, 0:Tn * B_SH].rearrange("p (t b) -> p t b", t=Tn),
                in_=term_d[:])
            nc.vector.memset(ndf[0:1, Tn * B_SH:], 0.0)
            nc.vector.tensor_scalar(ndf, ndf, -1.0, 1.0,
                                    op0=ALU.mult, op1=ALU.add)
            nd3 = ndf.rearrange("p (t b) -> p t b", t=Tn + 1)

            # encoder weights stay resident
            wenc_sb = []
            for oc in range(4):
                wenc_t = cpool.tile([128, FEAT], RT, name=f"wenc{oc}")
                nc.sync.dma_start(out=wenc_t, in_=_r(wencT_d[oc]))
                wenc_sb.append(wenc_t)
            benc_sb = cpool.tile([128, 4], F32)
            nc.sync.dma_start(out=benc_sb, in_=bencT_d[:])
            b0p_sb = cpool.tile([128, 20], F32)
            nc.sync.dma_start(out=b0p_sb, in_=b0p_d[:])
            b1p_sb = cpool.tile([128, 20], F32)
            nc.sync.dma_start(out=b1p_sb, in_=b1p_d[:])

            # recurrence states (h double-buffered for pipelining)
            h0A = spool.tile([128, 160], F32)
            h0B = spool.tile([128, 160], F32)
            c0_st = spool.tile([128, 160], F32)
            h0p = spool.tile([128, 160], F32)
            c0p = spool.tile([128, 160], F32)
            h1A = spool.tile([128, 160], F32)
            h1B = spool.tile([128, 160], F32)
            c1_st = spool.tile([128, 160], F32)
            h1p = spool.tile([128, 160], F32)
            c1p = spool.tile([128, 160], F32)
            for st in (h0A, h0B, c0_st, h0p, c0p, h1A, h1B, c1_st, h1p, c1p):
                nc.vector.memset(st, 0.0)

            def load_w(src, tag, dt=F32):
                tiles = []
                for kc in range(5):
                    wt = wpool.tile([128, G4], dt, name=f"w{tag}{kc}",
                                    tag=f"wmat{kc}")
                    nc.sync.dma_start(
                        out=wt, in_=src[kc] if dt == F32 else src[kc].bitcast(dt))
                    tiles.append(wt)
                return tiles

            # ---------------- phase 1+2: encoder + ihpre0 ----------------
            wih0_sb = load_w(wih0_d, "ih0", RT)
            with (
                tc.tile_pool(name="p12", bufs=2) as pool,
                tc.tile_pool(name="p12ps", bufs=4, space="PSUM") as pspool,
                tc.tile_pool(name="stage", bufs=1) as stpool,
            ):
                for ci in range(NCH):
                    t0 = ci * TS
                    # obsT tiles for this chunk
                    obsT = [pool.tile([128, TS * B_SH], RT,
                                      name=f"obsT{oc}",
                                      tag=f"obsT{oc}") for oc in range(4)]
                    for nt in range(4):
                        on = pool.tile([128, OBS], F32, name="on", tag="on")
                        nc.sync.dma_start(
                            out=on,
                            in_=obs_d[t0 + nt * 4: t0 + nt * 4 + 4].flatten_outer_dims())
                        for oc in range(4):
                            pst = pspool.tile([128, 128], F32, name="pst",
                                              tag="pst", bufs=2)
                            nc.tensor.transpose(
                                pst, on[:, oc * 128:(oc + 1) * 128], ident)
                            nc.vector.tensor_copy(
                                obsT[oc][:, nt * 128:(nt + 1) * 128], pst)
                    # encoder -> xk tiles
                    xk = [pool.tile([128, TS * B_SH], RT, name=f"xk{ft}",
                                    tag=f"xk{ft}") for ft in range(4)]
                    xk4 = pool.tile([16, TS * B_SH], F32, name="xk4", tag="xk4")
                    for ft in range(4):
                        psf = pspool.tile([128, TS * B_SH], F32, name="psf",
                                          tag="psf")
                        for oc in range(4):
                            nc.tensor.matmul(
                                psf,
                                _r(wenc_sb[oc][:, ft * 128:(ft + 1) * 128]),
                                _r(obsT[oc]), start=(oc == 0), stop=(oc == 3))
                        nc.scalar.activation(xk[ft], psf, AF.Relu,
                                             bias=benc_sb[:, ft:ft + 1])
                    # reward row -> xk4[0]
                    nc.sync.dma_start(
                        out=xk4[0:1].rearrange("p (t b) -> p t b", t=TS),
                        in_=rew_d[0:1, t0:t0 + TS, :])
                    nc.vector.tensor_scalar(
                        xk4[0:1], xk4[0:1], -1.0, 1.0,
                        op0=ALU.max, op1=ALU.min)
                    # one-hot rows -> xk4[1:16]
                    for nt in range(4):
                        laci = pool.tile([128, 1], I32, name="laci", tag="laci")
                        nc.sync.dma_start(
                            out=laci,
                            in_=la_d[t0 + nt * 4: t0 + nt * 4 + 4].flatten_outer_dims())
                        lac = pool.tile([128, 1], F32, name="lac", tag="lac")
                        nc.vector.tensor_copy(lac, laci)
                        ohn = pool.tile([128, 16], F32, name="ohn", tag="ohn")
                        nc.vector.tensor_scalar(ohn, iota16, lac, None,
                                                op0=ALU.is_equal)
                        pso = pspool.tile([16, 128], F32, name="pso",
                                          tag="pso", bufs=2)
                        nc.tensor.transpose(pso, ohn, ident)
                        oht = pool.tile([16, 128], F32, name="oht", tag="oht")
                        nc.vector.tensor_copy(oht, pso)
                        nc.sync.dma_start(
                            out=xk4[1:16, nt * 128:(nt + 1) * 128],
                            in_=oht[1:16])
                    xk4r = pool.tile([16, TS * B_SH], RT, name="xk4r",
                                     tag="xk4r")
                    nc.vector.tensor_copy(xk4r, xk4)
                    xks = xk + [xk4r]
                    # ihpre0 pieces
                    stage = stpool.tile([128, TS * 640], F32)
                    stv = stage.rearrange("p (t q) -> p t q", t=TS)
                    for pi in range(20):
                        msz, joff, col, _ = piece_meta(pi)
                        psp = pspool.tile([128, TS * B_SH], F32, name="psp",
                                          tag="psf")
                        for kc in range(5):
                            ksz = KC[kc][1]
                            nc.tensor.matmul(
                                psp[:msz],
                                _r(wih0_sb[kc][:ksz, joff:joff + msz]),
                                _r(xks[kc][:ksz] if kc == 4 else xks[kc]),
                                start=(kc == 0), stop=(kc == 4))
                        nc.scalar.add(
                            stv[:msz, :, col:col + 32],
                            psp[:msz].rearrange("p (t b) -> p t b", t=TS),
                            b0p_sb[:msz, pi:pi + 1])
                    for t in range(TS):
                        nc.sync.dma_start(
                            out=pre0[t0 + t][:, 0:512],
                            in_=stage[:, t * 640: t * 640 + 512])
                        nc.sync.dma_start(
                            out=pre0[t0 + t][0:16, 512:640],
                            in_=stage[0:16, t * 640 + 512:(t + 1) * 640])

            # ---------------- phase 3: layer-0 recurrence ----------------
            if not os.environ.get("K_SKIP_REC"):
                whh0_sb = load_w(whh0_d, "hh0")
                _recurrence(nc, tc, Tn, whh0_sb, pre0, h0a, h0A, h0B,
                            c0_st, h0p, c0p, nd3)

            # ---------------- phase 4: ihpre1 ----------------
            wih1_sb = load_w(wih1_d, "ih1", RT)
            _big_pre(nc, tc, NCH, wih1_sb, h0a, b1p_sb, pre1)

            # ---------------- phase 5: layer-1 recurrence ----------------
            if not os.environ.get("K_SKIP_REC"):
                whh1_sb = load_w(whh1_d, "hh1")
                _recurrence(nc, tc, Tn, whh1_sb, pre1, h1a, h1A, h1B,
                            c1_st, h1p, c1p, nd3)

            # ---------------- phase 6: heads + argmax ----------------
            head_sb = []
            for kc in range(5):
                ht = cpool.tile([128, 16], F32, name=f"head{kc}")
                nc.sync.dma_start(out=ht, in_=headT_d[kc])
                head_sb.append(ht)
            bh_sb = cpool.tile([16, 1], F32)
            nc.sync.dma_start(out=bh_sb, in_=bhead_d[:])
            with (
                tc.tile_pool(name="p6", bufs=2) as pool,
                tc.tile_pool(name="p6ps", bufs=4, space="PSUM") as pspool,
            ):
                for ci in range(NCH):
                    t0 = ci * TS
                    hk = []
                    for kc in range(5):
                        hkt = pool.tile([128, TS * B_SH], F32, name=f"hk{kc}",
                                        tag=f"hk{kc}")
                        nc.sync.dma_start(
                            out=hkt,
                            in_=h1a[t0:t0 + TS, :, kc * 32:(kc + 1) * 32]
                            .rearrange("t p b -> p t b"))
                        hk.append(hkt)
                    psl = pspool.tile([16, TS * B_SH], F32, name="psl",
                                      tag="psl")
                    for kc in range(5):
                        ksz = KC[kc][1]
                        nc.tensor.matmul(psl, head_sb[kc][:ksz], hk[kc][:ksz],
                                         start=(kc == 0), stop=(kc == 4))
                    lsb = pool.tile([16, TS * B_SH], F32, name="lsb", tag="lsb")
                    nc.scalar.add(lsb, psl, bh_sb[:, 0:1])
                    n0 = ci * TS * B_SH
                    nc.sync.dma_start(out=polT_d[:, n0:n0 + TS * B_SH],
                                      in_=lsb[0:A])
                    nc.sync.dma_start(out=baseT_d[:, n0:n0 + TS * B_SH],
                                      in_=lsb[A:16])
                    for nt in range(4):
                        psz = pspool.tile([128, 16], F32, name="psz", tag="psz")
                        nc.tensor.transpose(
                            psz, lsb[:, nt * 128:(nt + 1) * 128],
                            ident[0:16, 0:16])
                        zt = pool.tile([128, A], F32, name="zt", tag="zt")
                        gmt = pool.tile([128, A], F32, name="gmt", tag="gmt")
                        nc.sync.dma_start(
                            out=gmt,
                            in_=gum_d[n0 + nt * 128: n0 + (nt + 1) * 128])
                        nc.vector.tensor_add(zt, psz[:, 0:A], gmt)
                        mx = pool.tile([128, 8], F32, name="mx", tag="mx")
                        nc.vector.max(mx, zt)
                        idx = pool.tile([128, 8], U32, name="idx", tag="idx")
                        nc.vector.max_index(idx, mx, zt)
                        nc.sync.dma_start(
                            out=act_d[n0 + nt * 128: n0 + (nt + 1) * 128],
                            in_=idx[:, 0:1])

            # final states
            nc.sync.dma_start(out=hc_d[0], in_=h0p)
            nc.sync.dma_start(out=hc_d[1], in_=h1p)
            nc.sync.dma_start(out=hc_d[2], in_=c0p)
            nc.sync.dma_start(out=hc_d[3], in_=c1p)

    if not nc.is_finalized():
        nc.finalize()
    return nc


def _big_pre(nc, tc, NCH, w_sb, hsrc, bp_sb, pre_out):
    """ihpre = w'.T-pieces @ hT(+bias) over all chunks, written to DRAM."""
    with (
        tc.tile_pool(name="p4", bufs=2) as pool,
        tc.tile_pool(name="p4ps", bufs=4, space="PSUM") as pspool,
        tc.tile_pool(name="stage4", bufs=1) as stpool,
    ):
        for ci in range(NCH):
            t0 = ci * TS
            hk = []
            for kc in range(5):
                hkt = pool.tile([128, TS * B_SH], RT, name=f"hk{kc}",
                                tag=f"hk{kc}")
                nc.sync.dma_start(
                    out=hkt,
                    in_=_r(hsrc[t0:t0 + TS, :, kc * 32:(kc + 1) * 32]
                           .rearrange("t p b -> p t b")))
                hk.append(hkt)
            stage = stpool.tile([128, TS * 640], F32)
            stv = stage.rearrange("p (t q) -> p t q", t=TS)
            for pi in range(20):
                msz, joff, col, _ = piece_meta(pi)
                psp = pspool.tile([128, TS * B_SH], F32, name="psp", tag="psp")
                for kc in range(5):
                    ksz = KC[kc][1]
                    nc.tensor.matmul(
                        psp[:msz], _r(w_sb[kc][:ksz, joff:joff + msz]),
                        _r(hk[kc][:ksz]), start=(kc == 0), stop=(kc == 4))
                nc.scalar.add(
                    stv[:msz, :, col:col + 32],
                    psp[:msz].rearrange("p (t b) -> p t b", t=TS),
                    bp_sb[:msz, pi:pi + 1])
            for t in range(TS):
                nc.sync.dma_start(out=pre_out[t0 + t][:, 0:512],
                                  in_=stage[:, t * 640: t * 640 + 512])
                nc.sync.dma_start(out=pre_out[t0 + t][0:16, 512:640],
                                  in_=stage[0:16, t * 640 + 512:(t + 1) * 640])


def _recurrence(nc, tc, Tn, w_sb, pre_src, h_out, hA, hB, c_st, hp_st, cp_st,
                nd3):
    """Sequential LSTM layer.

    Pieces are chunk-major, so hidden chunk c's four gates live in cols
    [c*128, c*128+128) of the 640-wide gate layout. The main chunks
    (c=0..3) are processed as one batched elementwise group; the tail
    (16 rows) as a second, overlapping the next step's lead-in. The h
    state is double-buffered across steps and the done-mask of step t+1
    is folded into step t's state writes (nd3 has a trailing ones row).
    hp_st / cp_st keep the unmasked h/c for the kernel outputs.
    """
    with (
        tc.tile_pool(name="rec", bufs=2) as pool,
        tc.tile_pool(name="recps", bufs=2, space="PSUM") as pspool,
    ):
        ucnt = [0]

        def body(tv):
            u = ucnt[0] % 2
            ucnt[0] += 1
            h_rd, h_wr = (hA, hB) if u == 0 else (hB, hA)
            ndb = pool.tile([128, B_SH], F32, name="ndb", tag="ndb")
            nc.gpsimd.partition_broadcast(ndb, nd3[0:1, ds(tv + 1, 1), :])
            pre_t = pool.tile([128, 640], F32, name="pre_t", tag="pre_t")
            nc.sync.dma_start(
                out=pre_t[:, 0:512],
                in_=pre_src[ds(tv, 1)].flatten_outer_dims()[:, 0:512])
            nc.sync.dma_start(
                out=pre_t[0:16, 512:640],
                in_=pre_src[ds(tv, 1)].flatten_outer_dims()[0:16, 512:640])
            G = pspool.tile([128, 640], F32, name="G", tag="G")
            Gs = pool.tile([128, 640], F32, name="Gs", tag="Gs")

            def mms(pis):
                for pi in pis:
                    msz, joff, col, _ = piece_meta(pi)
                    for kc in range(5):
                        ksz = KC[kc][1]
                        nc.tensor.matmul(
                            G[:msz, col:col + 32],
                            w_sb[kc][:ksz, joff:joff + msz],
                            h_rd[:ksz, kc * 32:(kc + 1) * 32],
                            start=(kc == 0), stop=(kc == 4))

            def elemwise_main():
                nc.vector.tensor_add(Gs[:, 0:512], G[:, 0:512],
                                     pre_t[:, 0:512])
                gv = Gs[:, 0:512].rearrange("p (c q) -> p c q", c=4)
                iff, gg, oo = gv[:, :, 0:64], gv[:, :, 64:96], gv[:, :, 96:128]
                nc.scalar.activation(iff, iff, AF.Tanh, scale=0.5)
                nc.scalar.activation(gg, gg, AF.Tanh)
                nc.scalar.activation(oo, oo, AF.Tanh, scale=0.5)
                nc.vector.tensor_scalar(iff, iff, 0.5, 0.5,
                                        op0=ALU.mult, op1=ALU.add)
                nc.vector.tensor_scalar(oo, oo, 0.5, 0.5,
                                        op0=ALU.mult, op1=ALU.add)
                iv, fv = gv[:, :, 0:32], gv[:, :, 32:64]
                c3 = lambda t: t[:, 0:128].rearrange("p (c b) -> p c b", c=4)
                tmp = pool.tile([128, 128], F32, name="tmp", tag="tmp")
                th = pool.tile([128, 128], F32, name="th", tag="th")
                tm3 = tmp.rearrange("p (c b) -> p c b", c=4)
                nd4 = ndb.unsqueeze(1).broadcast_to([128, 4, B_SH])
                nc.vector.tensor_mul(tm3, fv, c3(c_st))
                nc.vector.tensor_mul(c3(cp_st), iv, gg)
                nc.vector.tensor_add(c3(cp_st), c3(cp_st), tm3)
                nc.scalar.activation(th, cp_st[:, 0:128], AF.Tanh)
                nc.vector.tensor_mul(
                    c3(hp_st), oo, th.rearrange("p (c b) -> p c b", c=4))
                nc.vector.tensor_mul(c3(h_wr), c3(hp_st), nd4)
                nc.vector.tensor_mul(c3(c_st), c3(cp_st), nd4)

            def elemwise_tail():
                nc.vector.tensor_add(Gs[0:16, 512:640], G[0:16, 512:640],
                                     pre_t[0:16, 512:640])
                iff = Gs[0:16, 512:576]
                gg = Gs[0:16, 576:608]
                oo = Gs[0:16, 608:640]
                nc.scalar.activation(iff, iff, AF.Tanh, scale=0.5)
                nc.scalar.activation(gg, gg, AF.Tanh)
                nc.scalar.activation(oo, oo, AF.Tanh, scale=0.5)
                nc.vector.tensor_scalar(iff, iff, 0.5, 0.5,
                                        op0=ALU.mult, op1=ALU.add)
                nc.vector.tensor_scalar(oo, oo, 0.5, 0.5,
                                        op0=ALU.mult, op1=ALU.add)
                iv, fv = Gs[0:16, 512:544], Gs[0:16, 544:576]
                st = (slice(0, 16), slice(128, 160))
                tmp = pool.tile([128, 32], F32, name="tmpt", tag="tmpt")
                th = pool.tile([128, 32], F32, name="tht", tag="tht")
                nc.vector.tensor_mul(tmp[0:16], fv, c_st[st])
                nc.vector.tensor_mul(cp_st[st], iv, gg)
                nc.vector.tensor_add(cp_st[st], cp_st[st], tmp[0:16])
                nc.scalar.activation(th[0:16], cp_st[st], AF.Tanh)
                nc.vector.tensor_mul(hp_st[st], oo, th[0:16])
                nc.vector.tensor_mul(h_wr[st], hp_st[st], ndb[0:16])
                nc.vector.tensor_mul(c_st[st], cp_st[st], ndb[0:16])

            mms(range(16))
            elemwise_main()
            mms(range(16, 20))
            elemwise_tail()
            # store unmasked h for later phases
            nc.sync.dma_start(out=h_out[ds(tv, 1)].flatten_outer_dims(),
                              in_=hp_st)

        tc.For_i_unrolled(0, Tn, 1, body, max_unroll=8)


# ---------------------------------------------------------------------------
# host wrapper
# ---------------------------------------------------------------------------

_PROG_CACHE = {}


def get_program(Tn=T_FULL):
    if Tn not in _PROG_CACHE:
        _PROG_CACHE[Tn] = build_program(Tn)
    return _PROG_CACHE[Tn]


def _gumbel_table(Tn, Bn):
    import jax
    import jax.numpy as jnp
    with jax.default_device(jax.devices("cpu")[0]):
        g = jax.random.gumbel(jax.random.key(1), (Tn, Bn, A), jnp.float32)
        return np.asarray(jax.device_get(g))


def make_in_maps(obs, last_action, reward, terminated,
                 W_enc, b_enc, w_ih0, w_hh0, b_ih0, b_hh0,
                 w_ih1, w_hh1, b_ih1, b_hh1,
                 W_pol, b_pol, W_base, b_base):
    Tn, Bn = obs.shape[0], obs.shape[1]
    ncores = Bn // B_SH
    # shared packed weights
    wencT = np.zeros((4, 128, FEAT), np.float32)
    We = np.asarray(W_enc, np.float32)  # [FEAT, OBS]
    for oc in range(4):
        wencT[oc] = We[:, oc * 128:(oc + 1) * 128].T
    bencT = np.ascontiguousarray(
        np.asarray(b_enc, np.float32).reshape(4, 128).T)
    wih0T, whh0T = pack_wT(w_ih0), pack_wT(w_hh0)
    wih1T, whh1T = pack_wT(w_ih1), pack_wT(w_hh1)
    b0p = pack_bias(b_ih0, b_hh0)
    b1p = pack_bias(b_ih1, b_hh1)
    Wh = np.concatenate([np.asarray(W_pol, np.float32),
                         np.asarray(W_base, np.float32)], 0)  # [16, 528]
    headT = np.zeros((5, 128, 16), np.float32)
    for kc, (off, ksz) in enumerate(KC):
        headT[kc, :ksz, :] = Wh[:, off:off + ksz].T
    bhead = np.concatenate(
        [np.asarray(b_pol, np.float32), np.asarray(b_base, np.float32)]
    ).reshape(16, 1)
    gum = _gumbel_table(Tn, Bn)

    shared = dict(wencT=wencT, bencT=bencT, wih0T=wih0T, whh0T=whh0T,
                  wih1T=wih1T, whh1T=whh1T, b0p=b0p, b1p=b1p,
                  headT=headT, bhead=bhead)
    obs = np.asarray(obs, np.float32)
    la = np.asarray(last_action, np.int32)
    rew = np.asarray(reward, np.float32)
    term = np.asarray(terminated).astype(np.uint8)
    in_maps = []
    for c in range(ncores):
        sl = slice(c * B_SH, (c + 1) * B_SH)
        in_maps.append(dict(
            obs=np.ascontiguousarray(obs[:, sl]),
            la=np.ascontiguousarray(la[:, sl])[..., None],
            rew=np.ascontiguousarray(rew[:, sl])[None],
            term=np.ascontiguousarray(term[:, sl])[None],
            gum=np.ascontiguousarray(gum[:, sl].reshape(Tn * B_SH, A)),
            **shared))
    return in_maps


def assemble_outputs(results, Tn):
    pol, base, act, hT, cT = [], [], [], [], []
    for r in results:
        pol.append(r["polT"].reshape(A, Tn, B_SH).transpose(1, 2, 0))
        base.append(r["baseT"].reshape(Tn, B_SH))
        act.append(r["act"].astype(np.int32).reshape(Tn, B_SH))
        hc = r["hc"].reshape(4, 128, 160)  # h0, h1, c0, c1
        hl, cl = [], []
        for l in range(2):
            hmat = np.zeros((B_SH, H), np.float32)
            cmat = np.zeros((B_SH, H), np.float32)
            for kc, (off, ksz) in enumerate(KC):
                hmat[:, off:off + ksz] = hc[l][:ksz, kc * 32:(kc + 1) * 32].T
                cmat[:, off:off + ksz] = hc[2 + l][:ksz,
                                                   kc * 32:(kc + 1) * 32].T
            hl.append(hmat)
            cl.append(cmat)
        hT.append(np.stack(hl))  # [2, 32, 528]
        cT.append(np.stack(cl))
    policy = np.concatenate(pol, axis=1)
    baseline = np.concatenate(base, axis=1)
    action = np.concatenate(act, axis=1)
    hT = np.concatenate(hT, axis=1)
    cT = np.concatenate(cT, axis=1)
    return policy, baseline, action, hT, cT


def kernel(**inputs):
    Tn = inputs["obs"].shape[0]
    nc = get_program(Tn)
    in_maps = make_in_maps(**inputs)
    res = run_bass_kernel_spmd(nc, in_maps, core_ids=list(range(len(in_maps))))
    return assemble_outputs(res.results, Tn)


# revision 22
# speedup vs baseline: 879.4772x; 879.4772x over previous
"""Trainium2 Bass kernel for DiscreteLSTMActor.

Strategy: data-parallel over batch (B=256 -> 32 per core x 8 cores).
Everything on-chip runs in a "transposed" layout: features on SBUF
partitions, samples along the free dim, so the LSTM gate elementwise
work uses all 128 lanes.

Per core pipeline:
  P1  encoder: PE-transpose obs tiles, featsT = relu(W_encT.T @ obsT + b),
      build xT = [featsT; clip(reward); one_hotT(last_action)]
  P2  ihpre0 = w_ih0'(permuted,transposed) @ xT + (b_ih0+b_hh0)  -> DRAM
  P3  layer-0 recurrence over T (only h-matmul per step, gate update in
      [128,32] chunk layout), h0 states -> DRAM
  P4  ihpre1 = w_ih1' @ h0T + biases -> DRAM
  P5  layer-1 recurrence, h1 states -> DRAM
  P6  heads: logitsT = headT.T @ h1T + b; argmax(logits+gumbel) on device
      (gumbel table for jax.random.key(1) is passed in as a constant)

Gate dimension (4H=2112) is permuted into 20 "pieces": 16 full pieces of
128 rows (gate-major: i0..i3 f0..f3 g0..g3 o0..o3) plus 4 tail pieces of
16 rows, so every elementwise slice is partition-aligned.

Host side only shards/replicates inputs, pre-packs weight layouts, and
re-assembles outputs.
"""

import os

import numpy as np

import concourse.bacc as bacc
import concourse.mybir as mybir
import concourse.tile as tile
from concourse.bass import ds
from concourse.bass_utils import run_bass_kernel_spmd
from concourse.masks import make_identity

F32 = mybir.dt.float32
I32 = mybir.dt.int32
U8 = mybir.dt.uint8
U32 = mybir.dt.uint32
F32R = mybir.dt.float32r
# fp32r (4x-rate fp32 PE path) crashes NRT on this stack; keep opt-in
USE_F32R = bool(int(os.environ.get("K_F32R", "0")))
RT = F32R if USE_F32R else F32
AF = mybir.ActivationFunctionType
ALU = mybir.AluOpType

T_FULL, B_FULL, OBS, FEAT, A = 256, 256, 512, 512, 15
H = FEAT + A + 1  # 528
G4 = 4 * H  # 2112
NCORES = 8
B_SH = B_FULL // NCORES  # 32
TS = 16  # timesteps per big-matmul chunk (chunk = TS*B_SH = 512 samples)

# k-chunks of the H=528 contraction dim
KC = [(0, 128), (128, 128), (256, 128), (384, 128), (512, 16)]
# gate pieces: (gate, chunk); 16 full + 4 tail
PIECES = [(g, c) for c in range(4) for g in range(4)] + [(g, 4) for g in range(4)]


def piece_meta(pi):
    g, c = PIECES[pi]
    msz = 128 if c < 4 else 16
    joff = pi * 128 if pi < 16 else 2048 + (pi - 16) * 16
    col = pi * 32  # column offset in the 640-wide gates/pre layout
    wrow = g * H + c * 128  # first row in the original [2112, 528] weight
    return msz, joff, col, wrow


def _jperm():
    idx = []
    for pi in range(20):
        msz, _, _, wrow = piece_meta(pi)
        idx.extend(range(wrow, wrow + msz))
    return np.asarray(idx, dtype=np.int64)


JPERM = _jperm()


def pack_wT(w):
    """w [2112, 528] -> [5, 128, 2112] (k-chunk, k-in-chunk, j')."""
    wp = np.asarray(w, np.float32)[JPERM, :]  # [2112, 528]
    out = np.zeros((5, 128, G4), np.float32)
    for kc, (off, ksz) in enumerate(KC):
        out[kc, :ksz, :] = wp[:, off:off + ksz].T
    return np.ascontiguousarray(out)


def pack_bias(b_ih, b_hh):
    bb = (np.asarray(b_ih, np.float32) + np.asarray(b_hh, np.float32))[JPERM]
    out = np.zeros((128, 20), np.float32)
    for pi in range(20):
        msz, joff, _, _ = piece_meta(pi)
        out[:msz, pi] = bb[joff:joff + msz]
    return np.ascontiguousarray(out)


# ---------------------------------------------------------------------------
# device program
# ---------------------------------------------------------------------------

def _r(ap):
    # fp32 tensors reinterpreted as float32r for the 4x-rate PE path on
    # big (N=512) matmuls; bit layout is identical
    return ap.bitcast(F32R) if USE_F32R else ap


def build_program(Tn):
    assert Tn % TS == 0
    NCH = Tn * B_SH // (TS * B_SH)  # number of 512-sample chunks = Tn // TS
    N = Tn * B_SH

    nc = bacc.Bacc("TRN2", target_bir_lowering=False)

    # inputs
    obs_d = nc.dram_tensor("obs", [Tn, B_SH, OBS], F32, kind="ExternalInput")
    la_d = nc.dram_tensor("la", [Tn, B_SH, 1], I32, kind="ExternalInput")
    rew_d = nc.dram_tensor("rew", [1, Tn, B_SH], F32, kind="ExternalInput")
    term_d = nc.dram_tensor("term", [1, Tn, B_SH], U8, kind="ExternalInput")
    gum_d = nc.dram_tensor("gum", [N, A], F32, kind="ExternalInput")
    wencT_d = nc.dram_tensor("wencT", [4, 128, FEAT], F32, kind="ExternalInput")
    bencT_d = nc.dram_tensor("bencT", [128, 4], F32, kind="ExternalInput")
    wih0_d = nc.dram_tensor("wih0T", [5, 128, G4], F32, kind="ExternalInput")
    whh0_d = nc.dram_tensor("whh0T", [5, 128, G4], F32, kind="ExternalInput")
    wih1_d = nc.dram_tensor("wih1T", [5, 128, G4], F32, kind="ExternalInput")
    whh1_d = nc.dram_tensor("whh1T", [5, 128, G4], F32, kind="ExternalInput")
    b0p_d = nc.dram_tensor("b0p", [128, 20], F32, kind="ExternalInput")
    b1p_d = nc.dram_tensor("b1p", [128, 20], F32, kind="ExternalInput")
    headT_d = nc.dram_tensor("headT", [5, 128, 16], F32, kind="ExternalInput")
    bhead_d = nc.dram_tensor("bhead", [16, 1], F32, kind="ExternalInput")

    # outputs
    polT_d = nc.dram_tensor("polT", [A, N], F32, kind="ExternalOutput")
    baseT_d = nc.dram_tensor("baseT", [1, N], F32, kind="ExternalOutput")
    act_d = nc.dram_tensor("act", [N, 1], U32, kind="ExternalOutput")
    hc_d = nc.dram_tensor("hc", [4, 128, 160], F32, kind="ExternalOutput")

    with tile.TileContext(nc) as tc:
        with (
            tc.tile_pool(name="dram", bufs=1, space="DRAM") as dpool,
            tc.tile_pool(name="const", bufs=1) as cpool,
            tc.tile_pool(name="wpool", bufs=1) as wpool,
            tc.tile_pool(name="state", bufs=1) as spool,
        ):
            pre0 = dpool.tile([Tn, 128, 640], F32)
            pre1 = dpool.tile([Tn, 128, 640], F32)
            h0a = dpool.tile([Tn, 128, 160], F32)
            h1a = dpool.tile([Tn, 128, 160], F32)

            ident = cpool.tile([128, 128], F32)
            make_identity(nc, ident)
            iota16i = cpool.tile([128, 16], I32)
            nc.gpsimd.iota(iota16i, pattern=[[1, 16]], base=-1,
                           channel_multiplier=0)
            iota16 = cpool.tile([128, 16], F32)
            nc.vector.tensor_copy(iota16, iota16i)

            # nd = 1 - terminated, with a trailing all-ones entry so the
            # recurrence can apply step t+1's mask at step t
            ndf = cpool.tile([1, (Tn + 1) * B_SH], F32)
            nc.gpsimd.dma_start(
                out=ndf[0:1, 0:Tn * B_SH].rearrange("p (t b) -> p t b", t=Tn),
                in_=term_d[:])
            nc.vector.memset(ndf[0:1, Tn * B_SH:], 0.0)
            nc.vector.tensor_scalar(ndf, ndf, -1.0, 1.0,
                                    op0=ALU.mult, op1=ALU.add)
            nd3 = ndf.rearrange("p (t b) -> p t b", t=Tn + 1)

            # encoder weights stay resident
            wenc_sb = []
            for oc in range(4):
                wenc_t = cpool.tile([128, FEAT], RT, name=f"wenc{oc}")
                nc.sync.dma_start(out=wenc_t, in_=_r(wencT_d[oc]))
                wenc_sb.append(wenc_t)
            benc_sb = cpool.tile([128, 4], F32)
            nc.sync.dma_start(out=benc_sb, in_=bencT_d[:])
            b0p_sb = cpool.tile([128, 20], F32)
            nc.sync.dma_start(out=b0p_sb, in_=b0p_d[:])
            b1p_sb = cpool.tile([128, 20], F32)
            nc.sync.dma_start(out=b1p_sb, in_=b1p_d[:])

            # recurrence states (h double-buffered for pipelining)
            h0A = spool.tile([128, 160], F32)
            h0B = spool.tile([128, 160], F32)
            c0_st = spool.tile([128, 160], F32)
            h0p = spool.tile([128, 160], F32)
            c0p = spool.tile([128, 160], F32)
            h1A = spool.tile([128, 160], F32)
            h1B = spool.tile([128, 160], F32)
            c1_st = spool.tile([128, 160], F32)
            h1p = spool.tile([128, 160], F32)
            c1p = spool.tile([128, 160], F32)
            for st in (h0A, h0B, c0_st, h0p, c0p, h1A, h1B, c1_st, h1p, c1p):
                nc.vector.memset(st, 0.0)

            def load_w(src, tag, dt=F32):
                tiles = []
                for kc in range(5):
                    wt = wpool.tile([128, G4], dt, name=f"w{tag}{kc}",
                                    tag=f"wmat{kc}")
                    nc.sync.dma_start(
                        out=wt, in_=src[kc] if dt == F32 else src[kc].bitcast(dt))
                    tiles.append(wt)
                return tiles

            # ---------------- phase 1+2: encoder + ihpre0 ----------------
            wih0_sb = load_w(wih0_d, "ih0", RT)
            with (
                tc.tile_pool(name="p12", bufs=2) as pool,
                tc.tile_pool(name="p12ps", bufs=4, space="PSUM") as pspool,
                tc.tile_pool(name="stage", bufs=1) as stpool,
            ):
                for ci in range(NCH):
                    t0 = ci * TS
                    # obsT tiles for this chunk
                    obsT = [pool.tile([128, TS * B_SH], RT,
                                      name=f"obsT{oc}",
                                      tag=f"obsT{oc}") for oc in range(4)]
                    for nt in range(4):
                        on = pool.tile([128, OBS], F32, name="on", tag="on")
                        nc.sync.dma_start(
                            out=on,
                            in_=obs_d[t0 + nt * 4: t0 + nt * 4 + 4].flatten_outer_dims())
                        for oc in range(4):
                            pst = pspool.tile([128, 128], F32, name="pst",
                                              tag="pst", bufs=2)
                            nc.tensor.transpose(
                                pst, on[:, oc * 128:(oc + 1) * 128], ident)
                            nc.vector.tensor_copy(
                                obsT[oc][:, nt * 128:(nt + 1) * 128], pst)
                    # encoder -> xk tiles
                    xk = [pool.tile([128, TS * B_SH], RT, name=f"xk{ft}",
                                    tag=f"xk{ft}") for ft in range(4)]
                    xk4 = pool.tile([16, TS * B_SH], F32, name="xk4", tag="xk4")
                    for ft in range(4):
                        psf = pspool.tile([128, TS * B_SH], F32, name="psf",
                                          tag="psf")
                        for oc in range(4):
                            nc.tensor.matmul(
                                psf,
                                _r(wenc_sb[oc][:, ft * 128:(ft + 1) * 128]),
                                _r(obsT[oc]), start=(oc == 0), stop=(oc == 3))
                        nc.scalar.activation(xk[ft], psf, AF.Relu,
                                             bias=benc_sb[:, ft:ft + 1])
                    # reward row -> xk4[0]
                    nc.sync.dma_start(
                        out=xk4[0:1].rearrange("p (t b) -> p t b", t=TS),
                        in_=rew_d[0:1, t0:t0 + TS, :])
                    nc.vector.tensor_scalar(
                        xk4[0:1], xk4[0:1], -1.0, 1.0,
                        op0=ALU.max, op1=ALU.min)
                    # one-hot rows -> xk4[1:16]
                    for nt in range(4):
                        laci = pool.tile([128, 1], I32, name="laci", tag="laci")
                        nc.sync.dma_start(
                            out=laci,
                            in_=la_d[t0 + nt * 4: t0 + nt * 4 + 4].flatten_outer_dims())
                        lac = pool.tile([128, 1], F32, name="lac", tag="lac")
                        nc.vector.tensor_copy(lac, laci)
                        ohn = pool.tile([128, 16], F32, name="ohn", tag="ohn")
                        nc.vector.tensor_scalar(ohn, iota16, lac, None,
                                                op0=ALU.is_equal)
                        pso = pspool.tile([16, 128], F32, name="pso",
                                          tag="pso", bufs=2)
                        nc.tensor.transpose(pso, ohn, ident)
                        oht = pool.tile([16, 128], F32, name="oht", tag="oht")
                        nc.vector.tensor_copy(oht, pso)
                        nc.sync.dma_start(
                            out=xk4[1:16, nt * 128:(nt + 1) * 128],
                            in_=oht[1:16])
                    xk4r = pool.tile([16, TS * B_SH], RT, name="xk4r",
                                     tag="xk4r")
                    nc.vector.tensor_copy(xk4r, xk4)
                    xks = xk + [xk4r]
                    # ihpre0 pieces
                    stage = stpool.tile([128, TS * 640], F32)
                    stv = stage.rearrange("p (t q) -> p t q", t=TS)
                    for pi in range(20):
                        msz, joff, col, _ = piece_meta(pi)
                        psp = pspool.tile([128, TS * B_SH], F32, name="psp",
                                          tag="psf")
                        for kc in range(5):
                            ksz = KC[kc][1]
                            nc.tensor.matmul(
                                psp[:msz],
                                _r(wih0_sb[kc][:ksz, joff:joff + msz]),
                                _r(xks[kc][:ksz] if kc == 4 else xks[kc]),
                                start=(kc == 0), stop=(kc == 4))
                        nc.scalar.add(
                            stv[:msz, :, col:col + 32],
                            psp[:msz].rearrange("p (t b) -> p t b", t=TS),
                            b0p_sb[:msz, pi:pi + 1])
                    for t in range(TS):
                        nc.sync.dma_start(
                            out=pre0[t0 + t][:, 0:512],
                            in_=stage[:, t * 640: t * 640 + 512])
                        nc.sync.dma_start(
                            out=pre0[t0 + t][0:16, 512:640],
                            in_=stage[0:16, t * 640 + 512:(t + 1) * 640])

            # ---------------- phase 3: layer-0 recurrence ----------------
            if not os.environ.get("K_SKIP_REC"):
                whh0_sb = load_w(whh0_d, "hh0")
                _recurrence(nc, tc, Tn, whh0_sb, pre0, h0a, h0A, h0B,
                            c0_st, h0p, c0p, nd3)

            # ---------------- phase 4: ihpre1 ----------------
            wih1_sb = load_w(wih1_d, "ih1", RT)
            _big_pre(nc, tc, NCH, wih1_sb, h0a, b1p_sb, pre1)

            # ---------------- phase 5: layer-1 recurrence ----------------
            if not os.environ.get("K_SKIP_REC"):
                whh1_sb = load_w(whh1_d, "hh1")
                _recurrence(nc, tc, Tn, whh1_sb, pre1, h1a, h1A, h1B,
                            c1_st, h1p, c1p, nd3)

            # ---------------- phase 6: heads + argmax ----------------
            head_sb = []
            for kc in range(5):
                ht = cpool.tile([128, 16], F32, name=f"head{kc}")
                nc.sync.dma_start(out=ht, in_=headT_d[kc])
                head_sb.append(ht)
            bh_sb = cpool.tile([16, 1], F32)
            nc.sync.dma_start(out=bh_sb, in_=bhead_d[:])
            with (
                tc.tile_pool(name="p6", bufs=2) as pool,
                tc.tile_pool(name="p6ps", bufs=4, space="PSUM") as pspool,
            ):
                for ci in range(NCH):
                    t0 = ci * TS
                    hk = []
                    for kc in range(5):
                        hkt = pool.tile([128, TS * B_SH], F32, name=f"hk{kc}",
                                        tag=f"hk{kc}")
                        nc.sync.dma_start(
                            out=hkt,
                            in_=h1a[t0:t0 + TS, :, kc * 32:(kc + 1) * 32]
                            .rearrange("t p b -> p t b"))
                        hk.append(hkt)
                    psl = pspool.tile([16, TS * B_SH], F32, name="psl",
                                      tag="psl")
                    for kc in range(5):
                        ksz = KC[kc][1]
                        nc.tensor.matmul(psl, head_sb[kc][:ksz], hk[kc][:ksz],
                                         start=(kc == 0), stop=(kc == 4))
                    lsb = pool.tile([16, TS * B_SH], F32, name="lsb", tag="lsb")
                    nc.scalar.add(lsb, psl, bh_sb[:, 0:1])
                    n0 = ci * TS * B_SH
                    nc.sync.dma_start(out=polT_d[:, n0:n0 + TS * B_SH],
                                      in_=lsb[0:A])
                    nc.sync.dma_start(out=baseT_d[:, n0:n0 + TS * B_SH],
                                      in_=lsb[A:16])
                    for nt in range(4):
                        psz = pspool.tile([128, 16], F32, name="psz", tag="psz")
                        nc.tensor.transpose(
                            psz, lsb[:, nt * 128:(nt + 1) * 128],
                            ident[0:16, 0:16])
                        zt = pool.tile([128, A], F32, name="zt", tag="zt")
                        gmt = pool.tile([128, A], F32, name="gmt", tag="gmt")
                        nc.sync.dma_start(
                            out=gmt,
                            in_=gum_d[n0 + nt * 128: n0 + (nt + 1) * 128])
                        nc.vector.tensor_add(zt, psz[:, 0:A], gmt)
                        mx = pool.tile([128, 8], F32, name="mx", tag="mx")
                        nc.vector.max(mx, zt)
                        idx = pool.tile([128, 8], U32, name="idx", tag="idx")
                        nc.vector.max_index(idx, mx, zt)
                        nc.sync.dma_start(
                            out=act_d[n0 + nt * 128: n0 + (nt + 1) * 128],
                            in_=idx[:, 0:1])

            # final states
            nc.sync.dma_start(out=hc_d[0], in_=h0p)
            nc.sync.dma_start(out=hc_d[1], in_=h1p)
            nc.sync.dma_start(out=hc_d[2], in_=c0p)
            nc.sync.dma_start(out=hc_d[3], in_=c1p)

    if not nc.is_finalized():
        nc.finalize()
    return nc


def _big_pre(nc, tc, NCH, w_sb, hsrc, bp_sb, pre_out):
    """ihpre = w'.T-pieces @ hT(+bias) over all chunks, written to DRAM."""
    with (
        tc.tile_pool(name="p4", bufs=2) as pool,
        tc.tile_pool(name="p4ps", bufs=4, space="PSUM") as pspool,
        tc.tile_pool(name="stage4", bufs=1) as stpool,
    ):
        for ci in range(NCH):
            t0 = ci * TS
            hk = []
            for kc in range(5):
                hkt = pool.tile([128, TS * B_SH], RT, name=f"hk{kc}",
                                tag=f"hk{kc}")
                nc.sync.dma_start(
                    out=hkt,
                    in_=_r(hsrc[t0:t0 + TS, :, kc * 32:(kc + 1) * 32]
                           .rearrange("t p b -> p t b")))
                hk.append(hkt)
            stage = stpool.tile([128, TS * 640], F32)
            stv = stage.rearrange("p (t q) -> p t q", t=TS)
            for pi in range(20):
                msz, joff, col, _ = piece_meta(pi)
                psp = pspool.tile([128, TS * B_SH], F32, name="psp", tag="psp")
                for kc in range(5):
                    ksz = KC[kc][1]
                    nc.tensor.matmul(
                        psp[:msz], _r(w_sb[kc][:ksz, joff:joff + msz]),
                        _r(hk[kc][:ksz]), start=(kc == 0), stop=(kc == 4))
                nc.scalar.add(
                    stv[:msz, :, col:col + 32],
                    psp[:msz].rearrange("p (t b) -> p t b", t=TS),
                    bp_sb[:msz, pi:pi + 1])
            for t in range(TS):
                nc.sync.dma_start(out=pre_out[t0 + t][:, 0:512],
                                  in_=stage[:, t * 640: t * 640 + 512])
                nc.sync.dma_start(out=pre_out[t0 + t][0:16, 512:640],
                                  in_=stage[0:16, t * 640 + 512:(t + 1) * 640])


def _recurrence(nc, tc, Tn, w_sb, pre_src, h_out, hA, hB, c_st, hp_st, cp_st,
                nd3):
    """Sequential LSTM layer.

    Pieces are chunk-major, so hidden chunk c's four gates live in cols
    [c*128, c*128+128) of the 640-wide gate layout. The main chunks
    (c=0..3) are processed as one batched elementwise group; the tail
    (16 rows) as a second, overlapping the next step's lead-in. The h
    state is double-buffered across steps and the done-mask of step t+1
    is folded into step t's state writes (nd3 has a trailing ones row).
    hp_st / cp_st keep the unmasked h/c for the kernel outputs.
    """
    with (
        tc.tile_pool(name="rec", bufs=2) as pool,
        tc.tile_pool(name="recps", bufs=2, space="PSUM") as pspool,
    ):
        ucnt = [0]

        def body(tv):
            u = ucnt[0] % 2
            ucnt[0] += 1
            h_rd, h_wr = (hA, hB) if u == 0 else (hB, hA)
            ndb = pool.tile([128, B_SH], F32, name="ndb", tag="ndb")
            nc.gpsimd.partition_broadcast(ndb, nd3[0:1, ds(tv + 1, 1), :])
            pre_t = pool.tile([128, 640], F32, name="pre_t", tag="pre_t")
            nc.sync.dma_start(
                out=pre_t[:, 0:512],
                in_=pre_src[ds(tv, 1)].flatten_outer_dims()[:, 0:512])
            nc.sync.dma_start(
                out=pre_t[0:16, 512:640],
                in_=pre_src[ds(tv, 1)].flatten_outer_dims()[0:16, 512:640])
            G = pspool.tile([128, 640], F32, name="G", tag="G")
            Gs = pool.tile([128, 640], F32, name="Gs", tag="Gs")

            def mms(pis):
                for pi in pis:
                    msz, joff, col, _ = piece_meta(pi)
                    for kc in range(5):
                        ksz = KC[kc][1]
                        nc.tensor.matmul(
                            G[:msz, col:col + 32],
                            w_sb[kc][:ksz, joff:joff + msz],
                            h_rd[:ksz, kc * 32:(kc + 1) * 32],
                            start=(kc == 0), stop=(kc == 4))

            def elemwise_chunks(c0, n):
                lo, hi = c0 * 128, (c0 + n) * 128
                nc.vector.tensor_add(Gs[:, lo:hi], G[:, lo:hi],
                                     pre_t[:, lo:hi])
                gv = Gs[:, lo:hi].rearrange("p (c q) -> p c q", c=n)
                iff, gg, oo = gv[:, :, 0:64], gv[:, :, 64:96], gv[:, :, 96:128]
                nc.scalar.activation(iff, iff, AF.Tanh, scale=0.5)
                nc.scalar.activation(gg, gg, AF.Tanh)
                nc.scalar.activation(oo, oo, AF.Tanh, scale=0.5)
                nc.vector.tensor_scalar(iff, iff, 0.5, 0.5,
                                        op0=ALU.mult, op1=ALU.add)
                nc.vector.tensor_scalar(oo, oo, 0.5, 0.5,
                                        op0=ALU.mult, op1=ALU.add)
                iv, fv = gv[:, :, 0:32], gv[:, :, 32:64]
                slo, shi = c0 * 32, (c0 + n) * 32
                c3 = lambda t: t[:, slo:shi].rearrange("p (c b) -> p c b", c=n)
                tmp = pool.tile([128, 128], F32, name="tmp", tag="tmp")
                th = pool.tile([128, 128], F32, name="th", tag="th")
                tm3 = tmp[:, 0:n * 32].rearrange("p (c b) -> p c b", c=n)
                nd4 = ndb.unsqueeze(1).broadcast_to([128, n, B_SH])
                nc.vector.tensor_mul(tm3, fv, c3(c_st))
                nc.vector.tensor_mul(c3(cp_st), iv, gg)
                nc.vector.tensor_add(c3(cp_st), c3(cp_st), tm3)
                nc.scalar.activation(th[:, 0:n * 32], cp_st[:, slo:shi],
                                     AF.Tanh)
                nc.vector.tensor_mul(
                    c3(hp_st), oo,
                    th[:, 0:n * 32].rearrange("p (c b) -> p c b", c=n))
                nc.vector.tensor_mul(c3(h_wr), c3(hp_st), nd4)
                nc.vector.tensor_mul(c3(c_st), c3(cp_st), nd4)

            def elemwise_tail():
                nc.vector.tensor_add(Gs[0:16, 512:640], G[0:16, 512:640],
                                     pre_t[0:16, 512:640])
                iff = Gs[0:16, 512:576]
                gg = Gs[0:16, 576:608]
                oo = Gs[0:16, 608:640]
                nc.scalar.activation(iff, iff, AF.Tanh, scale=0.5)
                nc.scalar.activation(gg, gg, AF.Tanh)
                nc.scalar.activation(oo, oo, AF.Tanh, scale=0.5)
                nc.vector.tensor_scalar(iff, iff, 0.5, 0.5,
                                        op0=ALU.mult, op1=ALU.add)
                nc.vector.tensor_scalar(oo, oo, 0.5, 0.5,
                                        op0=ALU.mult, op1=ALU.add)
                iv, fv = Gs[0:16, 512:544], Gs[0:16, 544:576]
                st = (slice(0, 16), slice(128, 160))
                tmp = pool.tile([128, 32], F32, name="tmpt", tag="tmpt")
                th = pool.tile([128, 32], F32, name="tht", tag="tht")
                nc.vector.tensor_mul(tmp[0:16], fv, c_st[st])
                nc.vector.tensor_mul(cp_st[st], iv, gg)
                nc.vector.tensor_add(cp_st[st], cp_st[st], tmp[0:16])
                nc.scalar.activation(th[0:16], cp_st[st], AF.Tanh)
                nc.vector.tensor_mul(hp_st[st], oo, th[0:16])
                nc.vector.tensor_mul(h_wr[st], hp_st[st], ndb[0:16])
                nc.vector.tensor_mul(c_st[st], cp_st[st], ndb[0:16])

            mms(range(8))
            elemwise_chunks(0, 2)
            mms(range(8, 16))
            elemwise_chunks(2, 2)
            mms(range(16, 20))
            elemwise_tail()
            # store unmasked h for later phases
            nc.sync.dma_start(out=h_out[ds(tv, 1)].flatten_outer_dims(),
                              in_=hp_st)

        tc.For_i_unrolled(0, Tn, 1, body, max_unroll=8)


# ---------------------------------------------------------------------------
# host wrapper
# ---------------------------------------------------------------------------

_PROG_CACHE = {}


def get_program(Tn=T_FULL):
    if Tn not in _PROG_CACHE:
        _PROG_CACHE[Tn] = build_program(Tn)
    return _PROG_CACHE[Tn]


def _gumbel_table(Tn, Bn):
    import jax
    import jax.numpy as jnp
    with jax.default_device(jax.devices("cpu")[0]):
        g = jax.random.gumbel(jax.random.key(1), (Tn, Bn, A), jnp.float32)
        return np.asarray(jax.device_get(g))


def make_in_maps(obs, last_action, reward, terminated,
                 W_enc, b_enc, w_ih0, w_hh0, b_ih0, b_hh0,
                 w_ih1, w_hh1, b_ih1, b_hh1,
                 W_pol, b_pol, W_base, b_base):
    Tn, Bn = obs.shape[0], obs.shape[1]
    ncores = Bn // B_SH
    # shared packed weights
    wencT = np.zeros((4, 128, FEAT), np.float32)
    We = np.asarray(W_enc, np.float32)  # [FEAT, OBS]
    for oc in range(4):
        wencT[oc] = We[:, oc * 128:(oc + 1) * 128].T
    bencT = np.ascontiguousarray(
        np.asarray(b_enc, np.float32).reshape(4, 128).T)
    wih0T, whh0T = pack_wT(w_ih0), pack_wT(w_hh0)
    wih1T, whh1T = pack_wT(w_ih1), pack_wT(w_hh1)
    b0p = pack_bias(b_ih0, b_hh0)
    b1p = pack_bias(b_ih1, b_hh1)
    Wh = np.concatenate([np.asarray(W_pol, np.float32),
                         np.asarray(W_base, np.float32)], 0)  # [16, 528]
    headT = np.zeros((5, 128, 16), np.float32)
    for kc, (off, ksz) in enumerate(KC):
        headT[kc, :ksz, :] = Wh[:, off:off + ksz].T
    bhead = np.concatenate(
        [np.asarray(b_pol, np.float32), np.asarray(b_base, np.float32)]
    ).reshape(16, 1)
    gum = _gumbel_table(Tn, Bn)

    shared = dict(wencT=wencT, bencT=bencT, wih0T=wih0T, whh0T=whh0T,
                  wih1T=wih1T, whh1T=whh1T, b0p=b0p, b1p=b1p,
                  headT=headT, bhead=bhead)
    obs = np.asarray(obs, np.float32)
    la = np.asarray(last_action, np.int32)
    rew = np.asarray(reward, np.float32)
    term = np.asarray(terminated).astype(np.uint8)
    in_maps = []
    for c in range(ncores):
        sl = slice(c * B_SH, (c + 1) * B_SH)
        in_maps.append(dict(
            obs=np.ascontiguousarray(obs[:, sl]),
            la=np.ascontiguousarray(la[:, sl])[..., None],
            rew=np.ascontiguousarray(rew[:, sl])[None],
            term=np.ascontiguousarray(term[:, sl])[None],
            gum=np.ascontiguousarray(gum[:, sl].reshape(Tn * B_SH, A)),
            **shared))
    return in_maps


def assemble_outputs(results, Tn):
    pol, base, act, hT, cT = [], [], [], [], []
    for r in results:
        pol.append(r["polT"].reshape(A, Tn, B_SH).transpose(1, 2, 0))
        base.append(r["baseT"].reshape(Tn, B_SH))
        act.append(r["act"].astype(np.int32).reshape(Tn, B_SH))
        hc = r["hc"].reshape(4, 128, 160)  # h0, h1, c0, c1
        hl, cl = [], []
        for l in range(2):
            hmat = np.zeros((B_SH, H), np.float32)
            cmat = np.zeros((B_SH, H), np.float32)
            for kc, (off, ksz) in enumerate(KC):
                hmat[:, off:off + ksz] = hc[l][:ksz, kc * 32:(kc + 1) * 32].T
                cmat[:, off:off + ksz] = hc[2 + l][:ksz,
                                                   kc * 32:(kc + 1) * 32].T
            hl.append(hmat)
            cl.append(cmat)
        hT.append(np.stack(hl))  # [2, 32, 528]
        cT.append(np.stack(cl))
    policy = np.concatenate(pol, axis=1)
    baseline = np.concatenate(base, axis=1)
    action = np.concatenate(act, axis=1)
    hT = np.concatenate(hT, axis=1)
    cT = np.concatenate(cT, axis=1)
    return policy, baseline, action, hT, cT


def kernel(**inputs):
    Tn = inputs["obs"].shape[0]
    nc = get_program(Tn)
    in_maps = make_in_maps(**inputs)
    res = run_bass_kernel_spmd(nc, in_maps, core_ids=list(range(len(in_maps))))
    return assemble_outputs(res.results, Tn)
